# revision 13
# baseline (speedup 1.0000x reference)
"""Trainium2 Bass kernel for nn_Encoder_Mamba (VMamba VSS encoder block).

Self-contained: kernel(**inputs) -> (down, skip) matching reference.py.

Sharding (8 cores): core c = (b=c//4, br=(c//2)%2, kg=c%2). Each core runs
the full VSS pipeline for branch (b, br), scanning its kg's two directions
(kg=0: row-major fwd+bwd; kg=1: col-major fwd+bwd). Cross-core collectives:
ReduceScatter(add) over pairs {2i,2i+1} (4-direction y sum), AllGather over
{0..3},{4..7} (channel concat). Phase C computed full-H per core; host keeps
cores 0 and 4. kg control differences are SPMD-uniform via 0/1 blend columns.

Selective scan: tensor_tensor_scan (state = dA*state + dBu), partitions =
(2 dirs x 64 ch), one scan per state n (16), chunked over L with carries.
"""
import sys

for _p in ('/opt/trn_rl_repo',):
    if _p not in sys.path:
        sys.path.insert(0, _p)

import numpy as np

B, C, OC, H, W = 2, 64, 128, 96, 96
HD, DI, N, R, K = 32, 64, 16, 2, 4
L = H * W              # 9216
TT = 512
NCH = L // TT          # 18
LH = L // 2            # 4608
NCH2 = LH // TT        # 9
EPS = 1e-5
HP, WP = H + 2, W + 2
RCH = 4
NRC = H // RCH         # 24
CW = RCH * W           # 384
LT = L // 128          # 72
LT2 = LH // 128        # 36

_CACHE = {}


def ts(i, size=TT):
    return slice(i * size, (i + 1) * size)


def _build_nc():
    import concourse.bacc as bacc
    import concourse.tile as tile
    from concourse import mybir
    from concourse.tile import add_dep_helper
    F32 = mybir.dt.float32
    AF = mybir.ActivationFunctionType
    OP = mybir.AluOpType
    AX = mybir.AxisListType

    nc = bacc.Bacc("TRN2", target_bir_lowering=False, debug=False, num_devices=8)
    dp = nc.declare_dram_parameter

    xb_d = dp("xb", [C, L], F32, isOutput=False)
    xbr_d = dp("xbr", [HD, L], F32, isOutput=False)
    dw33_lhsT_d = dp("dw33_lhsT", [HD, 9 * HD], F32, isOutput=False)
    dw33b_col_d = dp("dw33b_col", [HD, 1], F32, isOutput=False)
    in_lhsT_d = dp("in_lhsT", [HD, 2 * DI], F32, isOutput=False)
    inb_xp_col_d = dp("inb_xp_col", [DI, 1], F32, isOutput=False)
    inb_z_col_d = dp("inb_z_col", [DI, 1], F32, isOutput=False)
    conv_lhsT_d = dp("conv_lhsT", [DI, 9 * DI], F32, isOutput=False)
    convb_col_d = dp("convb_col", [DI, 1], F32, isOutput=False)
    dt_lhsT_d = dp("dt_lhsT", [DI, 128], F32, isOutput=False)
    dtb_col_d = dp("dtb_col", [128, 1], F32, isOutput=False)
    bc_lhsT_d = dp("bc_lhsT", [DI, 64], F32, isOutput=False)
    acols_d = dp("acols", [128, N], F32, isOutput=False)
    dmat_d = dp("dmat", [128, 128], F32, isOutput=False)
    onehot_d = dp("onehot", [64, N * 128], F32, isOutput=False)
    ident_d = dp("ident", [128, 128], F32, isOutput=False)
    out_lhsT_d = dp("out_lhsT", [DI, HD], F32, isOutput=False)
    onw_col_d = dp("onw_col", [DI, 1], F32, isOutput=False)
    onb_col_d = dp("onb_col", [DI, 1], F32, isOutput=False)
    ms1_col_d = dp("ms1_col", [HD, 1], F32, isOutput=False)
    gate_lhsT_d = dp("gate_lhsT", [128, C], F32, isOutput=False)
    gateb_col_d = dp("gateb_col", [C, 1], F32, isOutput=False)
    ax_lhsT_d = dp("ax_lhsT", [C, 5 * C], F32, isOutput=False)
    bnsc_col_d = dp("bnsc_col", [C, 1], F32, isOutput=False)
    bnsh_col_d = dp("bnsh_col", [C, 1], F32, isOutput=False)
    pw_lhsT_d = dp("pw_lhsT", [C, OC], F32, isOutput=False)
    pwb_col_d = dp("pwb_col", [OC, 1], F32, isOutput=False)
    msin_w_col_d = dp("msin_w_col", [C, 1], F32, isOutput=False)
    msin_b_col_d = dp("msin_b_col", [C, 1], F32, isOutput=False)
    sel_col_d = dp("sel_col", [128, 1], F32, isOutput=False)
    skip_d = dp("skip_out", [C, L], F32, isOutput=True)
    down_d = dp("down_out", [OC, 48 * 48], F32, isOutput=True)

    scr = nc.dram_tensor("ln_scr", [2, L], F32)       # phase-A LN stats
    scr2 = nc.dram_tensor("ln_scr2", [2, L], F32)     # phase-A LN r/m2
    scr3 = nc.dram_tensor("ln_scr3", [2, LH], F32)    # tail LN stats
    scr4 = nc.dram_tensor("ln_scr4", [2, LH], F32)    # tail LN r/m2
    bc_dr = nc.dram_tensor("bc_dr", [64, L], F32)
    z_half = nc.dram_tensor("z_half", [DI, LH], F32)
    xd_half = nc.dram_tensor("xd_half", [HD, LH], F32)
    rs_in = nc.dram_tensor("rs_in", [2 * DI, LH], F32)
    y_half_dr = nc.dram_tensor("y_half", [DI, LH], F32)
    ag_in = nc.dram_tensor("ag_in", [HD, LH], F32)
    xm_all = nc.dram_tensor("xm_all", [4 * HD, LH], F32)

    pairs = [[0, 1], [2, 3], [4, 5], [6, 7]]
    quads = [[0, 1, 2, 3], [4, 5, 6, 7]]

    tcm = tile.TileContext(nc)
    tc = tcm.__enter__()
    pools = []

    def mkpool(**kw):
        cm = tc.tile_pool(**kw)
        pools.append(cm)
        return cm.__enter__()

    cp = mkpool(name="const", bufs=1)
    lb = mkpool(name="Lb", bufs=3)
    w2 = mkpool(name="w2", bufs=2)
    w3 = mkpool(name="w3", bufs=3)
    sm = mkpool(name="sm", bufs=1)
    psA = mkpool(name="psA", bufs=2, space="PSUM")
    psB = mkpool(name="psB", bufs=2, space="PSUM")
    psY = mkpool(name="psY", bufs=1, space="PSUM")
    psZ = mkpool(name="psZ", bufs=1, space="PSUM")

    def cload(dram, shape):
        t = cp.tile(shape, F32, tag=f"c_{dram.name}")
        nc.sync.dma_start(t[:], dram[:])
        return t

    dw33_lhsT = cload(dw33_lhsT_d, [HD, 9 * HD])
    dw33b_col = cload(dw33b_col_d, [HD, 1])
    in_lhsT = cload(in_lhsT_d, [HD, 2 * DI])
    inb_xp_col = cload(inb_xp_col_d, [DI, 1])
    inb_z_col = cload(inb_z_col_d, [DI, 1])
    conv_lhsT = cload(conv_lhsT_d, [DI, 9 * DI])
    convb_col = cload(convb_col_d, [DI, 1])
    dt_lhsT = cload(dt_lhsT_d, [DI, 128])
    dtb_col = cload(dtb_col_d, [128, 1])
    bc_lhsT = cload(bc_lhsT_d, [DI, 64])
    acols = cload(acols_d, [128, N])
    dmat = cload(dmat_d, [128, 128])
    onehot = cload(onehot_d, [64, N * 128])
    ident = cload(ident_d, [128, 128])
    out_lhsT = cload(out_lhsT_d, [DI, HD])
    onw_col = cload(onw_col_d, [DI, 1])
    onb_col = cload(onb_col_d, [DI, 1])
    ms1_col = cload(ms1_col_d, [HD, 1])
    gate_lhsT = cload(gate_lhsT_d, [128, C])
    gateb_col = cload(gateb_col_d, [C, 1])
    ax_lhsT = cload(ax_lhsT_d, [C, 5 * C])
    bnsc_col = cload(bnsc_col_d, [C, 1])
    bnsh_col = cload(bnsh_col_d, [C, 1])
    pw_lhsT = cload(pw_lhsT_d, [C, OC])
    pwb_col = cload(pwb_col_d, [OC, 1])
    msin_w_col = cload(msin_w_col_d, [C, 1])
    msin_b_col = cload(msin_b_col_d, [C, 1])
    sel_col = cload(sel_col_d, [128, 1])
    ones1_32 = cp.tile([1, HD], F32)
    nc.vector.memset(ones1_32[:], 1.0)
    ones1_64 = cp.tile([1, DI], F32)
    nc.vector.memset(ones1_64[:], 1.0)
    o32_lhsT = cp.tile([HD, 1], F32)
    nc.vector.memset(o32_lhsT[:], 1.0 / HD)
    o64_lhsT = cp.tile([DI, 1], F32)
    nc.vector.memset(o64_lhsT[:], 1.0 / DI)
    carry = cp.tile([128, N], F32)
    nc.vector.memset(carry[:], 0.0)
    eps_col = cp.tile([128, 1], F32)
    nc.vector.memset(eps_col[:], EPS)
    one_col = cp.tile([128, 1], F32)
    nc.vector.memset(one_col[:], 1.0)

    def r3(t, h=HP):
        return t[:].rearrange("p (h w) -> p h w", h=h)

    # ============ PHASE A: dw33 + LN stats ============
    xpad32 = lb.tile([HD, HP * WP], F32, tag="Lb")
    nc.vector.memset(xpad32[:], 0.0)
    xbr_sb = lb.tile([HD, L], F32, tag="Lb")
    nc.sync.dma_start(xbr_sb[:], xbr_d[:])
    nc.scalar.copy(r3(xpad32)[:, 1:H + 1, 1:W + 1], r3(xbr_sb, h=H)[:, :, :])

    xd_sb = lb.tile([HD, L], F32, tag="Lb")
    stat_wr = []
    for rc in range(NRC):
        h0 = rc * RCH
        sl = slice(h0 * W, (h0 + RCH) * W)
        pc = psY.tile([HD, CW], F32, tag="y")
        for tap in range(9):
            dh, dw = tap // 3, tap % 3
            nc.tensor.matmul(pc[:], dw33_lhsT[:, HD * tap:HD * (tap + 1)],
                             r3(xpad32)[:, h0 + dh:h0 + dh + RCH, dw:dw + W],
                             start=(tap == 0), stop=(tap == 8))
        nc.scalar.activation(xd_sb[:, sl], pc[:], AF.Identity,
                             bias=dw33b_col[:, 0:1])
        x2 = w2.tile([HD, CW], F32, tag="w1")
        nc.scalar.activation(x2[:], xd_sb[:, sl], AF.Square)
        pstA = psA.tile([1, CW], F32, tag="a")
        nc.tensor.matmul(pstA[:], o32_lhsT[:], xd_sb[:, sl], start=True,
                         stop=True)
        pstB = psB.tile([1, CW], F32, tag="b")
        nc.tensor.matmul(pstB[:], o32_lhsT[:], x2[:], start=True, stop=True)
        sttA = w2.tile([1, CW], F32, tag="w2")
        nc.scalar.copy(sttA[:], pstA[:])
        sttB = w2.tile([1, CW], F32, tag="w3")
        nc.scalar.copy(sttB[:], pstB[:])
        stat_wr.append(nc.sync.dma_start(scr[0:1, sl], sttA[:]))
        stat_wr.append(nc.sync.dma_start(scr[1:2, sl], sttB[:]))

    # LN math in [128, LT] layout (via DRAM scratch)
    st_t = sm.tile([128, 2 * LT], F32, tag="stt")
    ld0 = nc.sync.dma_start(st_t[:, 0:LT],
                            scr[0:1, :].rearrange("o (p f) -> (o p) f", p=128))
    ld1 = nc.sync.dma_start(st_t[:, LT:2 * LT],
                            scr[1:2, :].rearrange("o (p f) -> (o p) f", p=128))
    for wr in stat_wr:
        add_dep_helper(ld0.ins, wr.ins, reason="scr RAW")
        add_dep_helper(ld1.ins, wr.ins, reason="scr RAW")
    musq = sm.tile([128, LT], F32, tag="t1")
    nc.scalar.activation(musq[:], st_t[:, 0:LT], AF.Square)
    var_t = sm.tile([128, LT], F32, tag="t2")
    nc.vector.tensor_tensor(var_t[:], st_t[:, LT:2 * LT], musq[:], OP.subtract)
    sd_t = sm.tile([128, LT], F32, tag="t3")
    nc.scalar.activation(sd_t[:], var_t[:], AF.Sqrt, bias=eps_col[:, 0:1])
    r_t = sm.tile([128, LT], F32, tag="t4")
    nc.vector.reciprocal(r_t[:], sd_t[:])
    m2_t = sm.tile([128, LT], F32, tag="t5")
    nc.vector.tensor_tensor(m2_t[:], st_t[:, 0:LT], r_t[:], OP.mult)
    w0 = nc.sync.dma_start(scr2[0:1, :].rearrange("o (p f) -> (o p) f", p=128),
                           r_t[:])
    w1_ = nc.sync.dma_start(scr2[1:2, :].rearrange("o (p f) -> (o p) f", p=128),
                            m2_t[:])

    # ============ in_proj -> z + xp(pad) ============
    xpad64 = lb.tile([DI, HP * WP], F32, tag="Lb")
    nc.vector.memset(xpad64[:], 0.0)
    zc_sb = lb.tile([DI, L], F32, tag="Lb")
    for rc in range(NRC):
        h0 = rc * RCH
        sl = slice(h0 * W, (h0 + RCH) * W)
        rr = w2.tile([1, CW], F32, tag="w1")
        mm = w2.tile([1, CW], F32, tag="w2")
        lr = nc.sync.dma_start(rr[:], scr2[0:1, sl])
        lm = nc.sync.dma_start(mm[:], scr2[1:2, sl])
        add_dep_helper(lr.ins, w0.ins, reason="scr2 RAW")
        add_dep_helper(lm.ins, w1_.ins, reason="scr2 RAW")
        rrep = psA.tile([HD, CW], F32, tag="a")
        nc.tensor.matmul(rrep[:], ones1_32[:], rr[:], start=True, stop=True)
        mrep = psB.tile([HD, CW], F32, tag="b")
        nc.tensor.matmul(mrep[:], ones1_32[:], mm[:], start=True, stop=True)
        xh = w3.tile([HD, CW], F32, tag="w6")
        nc.vector.tensor_tensor(xh[:], xd_sb[:, sl], rrep[:], OP.mult)
        nc.vector.tensor_tensor(xh[:], xh[:], mrep[:], OP.subtract)
        pxp = psY.tile([DI, CW], F32, tag="y")
        nc.tensor.matmul(pxp[:], in_lhsT[:, 0:DI], xh[:], start=True, stop=True)
        pzz = psB.tile([DI, CW], F32, tag="b")
        nc.tensor.matmul(pzz[:], in_lhsT[:, DI:2 * DI], xh[:], start=True,
                         stop=True)
        nc.scalar.activation(zc_sb[:, sl], pzz[:], AF.Identity,
                             bias=inb_z_col[:, 0:1])
        nc.scalar.activation(r3(xpad64)[:, 1 + h0:1 + h0 + RCH, 1:W + 1],
                             pxp[:], AF.Identity, bias=inb_xp_col[:, 0:1])

    # spill kg half of z and xd (blend by sel): half = h0 + sel*(h1-h0)
    z_wr, xd_wr = [], []
    for tci in range(NCH2):
        sl = ts(tci)
        t1 = w2.tile([DI, TT], F32, tag="w1")
        nc.vector.tensor_tensor(t1[:], zc_sb[:, LH + tci * TT:LH + (tci + 1) * TT],
                                zc_sb[:, sl], OP.subtract)
        nc.vector.scalar_tensor_tensor(t1[:], t1[:], sel_col[0:DI, 0:1],
                                       zc_sb[:, sl], OP.mult, OP.add)
        z_wr.append(nc.sync.dma_start(z_half[:, sl], t1[:]))
        t2 = w2.tile([HD, TT], F32, tag="w2")
        nc.vector.tensor_tensor(t2[:], xd_sb[:, LH + tci * TT:LH + (tci + 1) * TT],
                                xd_sb[:, sl], OP.subtract)
        nc.vector.scalar_tensor_tensor(t2[:], t2[:], sel_col[0:HD, 0:1],
                                       xd_sb[:, sl], OP.mult, OP.add)
        xd_wr.append(nc.sync.dma_start(xd_half[:, sl], t2[:]))

    # ============ conv3x3 + silu -> xc ============
    xc_sb = lb.tile([DI, L], F32, tag="Lb")
    for rc in range(NRC):
        h0 = rc * RCH
        pc = psY.tile([DI, CW], F32, tag="y")
        for tap in range(9):
            dh, dw = tap // 3, tap % 3
            nc.tensor.matmul(pc[:], conv_lhsT[:, DI * tap:DI * (tap + 1)],
                             r3(xpad64)[:, h0 + dh:h0 + dh + RCH, dw:dw + W],
                             start=(tap == 0), stop=(tap == 8))
        nc.scalar.activation(xc_sb[:, h0 * W:(h0 + RCH) * W], pc[:], AF.Silu,
                             bias=convb_col[:, 0:1])

    # ============ xs_s assembly ============
    xs_s = lb.tile([128, L], F32, tag="Lb")
    for rc in range(NRC):
        sl = slice(rc * CW, (rc + 1) * CW)
        cm = w2.tile([DI, CW], F32, tag="w1")
        # cm chunk: w in [4rc,4rc+4), h 0..95 -> xc[h*96+w]
        nc.vector.tensor_copy(
            cm[:].rearrange("p (w h) -> p w h", w=RCH),
            r3(xc_sb, h=H)[:, :, RCH * rc:RCH * (rc + 1)]
            .rearrange("p h w -> p w h"))
        nc.vector.tensor_tensor(cm[:], cm[:], xc_sb[:, sl], OP.subtract)
        nc.vector.scalar_tensor_tensor(xs_s[0:DI, sl], cm[:],
                                       sel_col[0:DI, 0:1], xc_sb[:, sl],
                                       OP.mult, OP.add)
    for rc in range(NRC):
        sl = slice(rc * CW, (rc + 1) * CW)
        rsl = slice(L - (rc + 1) * CW, L - rc * CW)
        rv = w2.tile([DI, CW], F32, tag="w2")
        nc.vector.tensor_copy(rv[:], xs_s[0:DI, rsl][:, ::-1])
        nc.sync.dma_start(xs_s[DI:128, sl], rv[:])

    # ============ projections: dt_s (SBUF), B/C -> DRAM ============
    dt_s = lb.tile([128, L], F32, tag="Lb")
    bc_wr = [None] * NCH
    for tci in range(NCH):
        sl = ts(tci)
        rsl = slice(L - (tci + 1) * TT, L - tci * TT)
        pdt = psA.tile([128, TT], F32, tag="a")
        nc.tensor.matmul(pdt[:], dt_lhsT[:], xs_s[0:DI, sl], start=True,
                         stop=True)
        edt = w2.tile([128, TT], F32, tag="w3")
        nc.scalar.activation(edt[:], pdt[:], AF.Exp, bias=dtb_col[:, 0:1])
        nc.scalar.activation(dt_s[0:DI, sl], edt[0:DI, :], AF.Ln,
                             bias=one_col[0:DI, 0:1])
        nc.scalar.activation(dt_s[DI:128, rsl][:, ::-1], edt[DI:128, :],
                             AF.Ln, bias=one_col[DI:128, 0:1])
        pbc = psB.tile([64, TT], F32, tag="b")
        nc.tensor.matmul(pbc[:], bc_lhsT[:], xs_s[0:DI, sl], start=True,
                         stop=True)
        bt = w2.tile([64, TT], F32, tag="w1")
        nc.scalar.copy(bt[0:32, :], pbc[0:32, :])
        nc.scalar.copy(bt[32:64, ::-1], pbc[32:64, :])
        wA = nc.sync.dma_start(bc_dr[0:32, sl], bt[0:32, :])
        wB = nc.sync.dma_start(bc_dr[32:64, rsl], bt[32:64, :])
        bc_wr[tci] = (wA, wB)

    # ============ PHASE B: scan ============
    y_all = lb.tile([DI, L], F32, tag="Lb")
    nc.vector.memset(y_all[:], 0.0)
    for ci in range(NCH):
        sl = ts(ci)
        rsl = slice(L - (ci + 1) * TT, L - ci * TT)
        bcc = w2.tile([64, TT], F32, tag="w2")
        lb1 = nc.sync.dma_start(bcc[0:16, :], bc_dr[0:16, sl])
        lb2 = nc.sync.dma_start(bcc[16:32, :], bc_dr[32:48, sl])
        lb3 = nc.sync.dma_start(bcc[32:48, :], bc_dr[16:32, sl])
        lb4 = nc.sync.dma_start(bcc[48:64, :], bc_dr[48:64, sl])
        for ld in (lb1, lb2, lb3, lb4):
            for wrs in (bc_wr[ci], bc_wr[NCH - 1 - ci]):
                for wr in wrs:
                    add_dep_helper(ld.ins, wr.ins, reason="bc RAW")
        dtu = w2.tile([128, TT], F32, tag="w1")
        nc.vector.tensor_tensor(dtu[:], dt_s[:, sl], xs_s[:, sl], OP.mult)
        ysum0 = psY.tile([DI, TT], F32, tag="y0")
        ysum1 = psZ.tile([DI, TT], F32, tag="y1")
        nc.tensor.matmul(ysum0[:], dmat[:, 0:DI], xs_s[:, sl], start=True,
                         stop=False)
        nc.tensor.matmul(ysum1[:], dmat[:, DI:128], xs_s[:, sl], start=True,
                         stop=False)
        for n in range(N):
            dA = w3.tile([128, TT], F32, tag="w6")
            nc.scalar.activation(dA[:], dt_s[:, sl], AF.Exp,
                                 scale=acols[:, n:n + 1])
            bps = psA.tile([128, TT], F32, tag="a")
            nc.tensor.matmul(bps[:], onehot[0:32, 128 * n:128 * (n + 1)],
                             bcc[0:32, :], start=True, stop=True)
            dBu = w3.tile([128, TT], F32, tag="w7")
            nc.vector.tensor_tensor(dBu[:], dtu[:], bps[:], OP.mult)
            hh = w3.tile([128, TT], F32, tag="w8")
            nc.vector.tensor_tensor_scan(hh[:], dA[:], dBu[:],
                                         carry[:, n:n + 1], OP.mult, OP.add)
            nc.scalar.copy(carry[:, n:n + 1], hh[:, TT - 1:TT])
            cps = psB.tile([128, TT], F32, tag="b")
            nc.tensor.matmul(cps[:], onehot[32:64, 128 * n:128 * (n + 1)],
                             bcc[32:64, :], start=True, stop=True)
            hc = w3.tile([128, TT], F32, tag="w9")
            nc.vector.tensor_tensor(hc[:], hh[:], cps[:], OP.mult)
            nc.tensor.matmul(ysum0[:], ident[:, 0:DI], hc[:], start=False,
                             stop=(n == N - 1))
            nc.tensor.matmul(ysum1[:], ident[:, DI:128], hc[:], start=False,
                             stop=(n == N - 1))
        nc.vector.tensor_tensor(y_all[:, sl], y_all[:, sl], ysum0[:], OP.add)
        nc.vector.tensor_tensor(y_all[:, rsl], y_all[:, rsl], ysum1[:, ::-1],
                                OP.add)

    # rs_in assembly: untranspose blend per 4-row chunk
    rs_wr = []
    for rc in range(NRC):
        sl = slice(rc * CW, (rc + 1) * CW)
        t1 = w2.tile([DI, CW], F32, tag="w1")
        nc.vector.tensor_copy(
            t1[:].rearrange("p (h w) -> p h w", h=RCH),
            y_all[:].rearrange("p (w h) -> p h w", w=W)
            [:, RCH * rc:RCH * (rc + 1), :])
        nc.vector.tensor_tensor(t1[:], t1[:], y_all[:, sl], OP.subtract)
        t2 = w2.tile([DI, CW], F32, tag="w2")
        nc.vector.scalar_tensor_tensor(t2[:], t1[:], sel_col[0:DI, 0:1],
                                       y_all[:, sl], OP.mult, OP.add)
        if rc < NRC // 2:
            rs_wr.append(nc.sync.dma_start(rs_in[0:DI, sl], t2[:]))
        else:
            sl2 = slice(rc * CW - LH, (rc + 1) * CW - LH)
            rs_wr.append(nc.sync.dma_start(rs_in[DI:2 * DI, sl2], t2[:]))
    cc1 = nc.gpsimd.collective_compute(
        "ReduceScatter", OP.add, replica_groups=pairs, ins=[rs_in[:]],
        outs=[y_half_dr[:]])

    # ============ B-tail: out-LN + gate + out-proj ============
    yh = lb.tile([DI, LH], F32, tag="Lb")
    lyh = nc.sync.dma_start(yh[:], y_half_dr[:])
    add_dep_helper(lyh.ins, cc1.ins, reason="RS RAW")
    st2_wr = []
    for tci in range(NCH2):
        sl = ts(tci)
        y2 = w2.tile([DI, TT], F32, tag="w1")
        nc.scalar.activation(y2[:], yh[:, sl], AF.Square)
        pstA = psA.tile([1, TT], F32, tag="a")
        nc.tensor.matmul(pstA[:], o64_lhsT[:], yh[:, sl], start=True, stop=True)
        pstB = psB.tile([1, TT], F32, tag="b")
        nc.tensor.matmul(pstB[:], o64_lhsT[:], y2[:], start=True, stop=True)
        stt2A = w2.tile([1, TT], F32, tag="w2")
        nc.scalar.copy(stt2A[:], pstA[:])
        stt2B = w2.tile([1, TT], F32, tag="w3")
        nc.scalar.copy(stt2B[:], pstB[:])
        st2_wr.append(nc.sync.dma_start(scr3[0:1, sl], stt2A[:]))
        st2_wr.append(nc.sync.dma_start(scr3[1:2, sl], stt2B[:]))
    st2t = sm.tile([128, 2 * LT2], F32, tag="stt")
    l20 = nc.sync.dma_start(st2t[:, 0:LT2],
                            scr3[0:1, :].rearrange("o (p f) -> (o p) f", p=128))
    l21 = nc.sync.dma_start(st2t[:, LT2:2 * LT2],
                            scr3[1:2, :].rearrange("o (p f) -> (o p) f", p=128))
    for wr in st2_wr:
        add_dep_helper(l20.ins, wr.ins, reason="scr3 RAW")
        add_dep_helper(l21.ins, wr.ins, reason="scr3 RAW")
    musq2 = sm.tile([128, LT2], F32, tag="t1")
    nc.scalar.activation(musq2[:], st2t[:, 0:LT2], AF.Square)
    var2 = sm.tile([128, LT2], F32, tag="t2")
    nc.vector.tensor_tensor(var2[:], st2t[:, LT2:2 * LT2], musq2[:],
                            OP.subtract)
    sd2 = sm.tile([128, LT2], F32, tag="t3")
    nc.scalar.activation(sd2[:], var2[:], AF.Sqrt, bias=eps_col[:, 0:1])
    r2 = sm.tile([128, LT2], F32, tag="t4")
    nc.vector.reciprocal(r2[:], sd2[:])
    m22 = sm.tile([128, LT2], F32, tag="t5")
    nc.vector.tensor_tensor(m22[:], st2t[:, 0:LT2], r2[:], OP.mult)
    w20 = nc.sync.dma_start(scr4[0:1, :].rearrange("o (p f) -> (o p) f", p=128),
                            r2[:])
    w21 = nc.sync.dma_start(scr4[1:2, :].rearrange("o (p f) -> (o p) f", p=128),
                            m22[:])

    zh2 = lb.tile([DI, LH], F32, tag="Lb")
    lz = nc.sync.dma_start(zh2[:], z_half[:])
    for wr in z_wr:
        add_dep_helper(lz.ins, wr.ins, reason="z RAW")
    xdh2 = lb.tile([HD, LH], F32, tag="Lb")
    lx = nc.sync.dma_start(xdh2[:], xd_half[:])
    for wr in xd_wr:
        add_dep_helper(lx.ins, wr.ins, reason="xd RAW")
    ag_wr = []
    for tci in range(NCH2):
        sl = ts(tci)
        rr = w2.tile([1, TT], F32, tag="w1")
        mm = w2.tile([1, TT], F32, tag="w2")
        lr = nc.sync.dma_start(rr[:], scr4[0:1, sl])
        lm = nc.sync.dma_start(mm[:], scr4[1:2, sl])
        add_dep_helper(lr.ins, w20.ins, reason="scr4 RAW")
        add_dep_helper(lm.ins, w21.ins, reason="scr4 RAW")
        rrep = psA.tile([DI, TT], F32, tag="a")
        nc.tensor.matmul(rrep[:], ones1_64[:], rr[:], start=True, stop=True)
        mrep = psB.tile([DI, TT], F32, tag="b")
        nc.tensor.matmul(mrep[:], ones1_64[:], mm[:], start=True, stop=True)
        t1 = w3.tile([DI, TT], F32, tag="w6")
        nc.vector.tensor_tensor(t1[:], yh[:, sl], rrep[:], OP.mult)
        nc.vector.tensor_tensor(t1[:], t1[:], mrep[:], OP.subtract)
        yl = w3.tile([DI, TT], F32, tag="w7")
        nc.scalar.activation(yl[:], t1[:], AF.Identity, scale=onw_col[:, 0:1],
                             bias=onb_col[:, 0:1])
        sz = w3.tile([DI, TT], F32, tag="w8")
        nc.scalar.activation(sz[:], zh2[:, sl], AF.Silu)
        nc.vector.tensor_tensor(yl[:], yl[:], sz[:], OP.mult)
        p32 = psY.tile([HD, TT], F32, tag="y")
        nc.tensor.matmul(p32[:], out_lhsT[:], yl[:], start=True, stop=True)
        piece = w3.tile([HD, TT], F32, tag="w9")
        nc.vector.scalar_tensor_tensor(piece[:], xdh2[:, sl], ms1_col[:, 0:1],
                                       p32[:], OP.mult, OP.add)
        ag_wr.append(nc.sync.dma_start(ag_in[:, sl], piece[:]))
    cc2 = nc.gpsimd.collective_compute(
        "AllGather", OP.bypass, replica_groups=quads, ins=[ag_in[:]],
        outs=[xm_all[:]])

    # ============ PHASE C (full-H, duplicated per b) ============
    xmp = lb.tile([C, (H + 2) * W], F32, tag="Lb")
    nc.vector.memset(xmp[:], 0.0)
    for rr_ in range(4):
        br, kg = (rr_ // 2) % 2, rr_ % 2
        lxm = nc.sync.dma_start(
            xmp[HD * br:HD * (br + 1), W + kg * LH:W + (kg + 1) * LH],
            xm_all[HD * rr_:HD * (rr_ + 1), :])
        add_dep_helper(lxm.ins, cc2.ins, reason="AG RAW")
    xbf = lb.tile([C, L], F32, tag="Lb")
    nc.sync.dma_start(xbf[:], xb_d[:])

    ssum = sm.tile([C, 1], F32, tag="c1")
    nc.vector.tensor_reduce(ssum[:], xmp[:, W:W + L], AX.X, OP.add)
    ssq = sm.tile([C, 1], F32, tag="c2")
    for qq in range(NCH):
        scr_t = w2.tile([C, TT], F32, tag="w1")
        if qq:
            part = sm.tile([C, 1], F32, tag="c3")
        else:
            part = ssq
        nc.scalar.activation(scr_t[:], xmp[:, W + qq * TT:W + (qq + 1) * TT],
                             AF.Square, accum_out=part[:, 0:1])
        if qq:
            nc.vector.tensor_tensor(ssq[:], ssq[:], part[:], OP.add)
    mu_c = sm.tile([C, 1], F32, tag="c4")
    nc.scalar.mul(mu_c[:], ssum[:], 1.0 / L)
    ex2 = sm.tile([C, 1], F32, tag="c5")
    nc.scalar.mul(ex2[:], ssq[:], 1.0 / L)
    musq_c = sm.tile([C, 1], F32, tag="c6")
    nc.scalar.activation(musq_c[:], mu_c[:], AF.Square)
    var_c = sm.tile([C, 1], F32, tag="c7")
    nc.vector.tensor_tensor(var_c[:], ex2[:], musq_c[:], OP.subtract)
    sd_c = sm.tile([C, 1], F32, tag="c8")
    nc.scalar.activation(sd_c[:], var_c[:], AF.Sqrt, bias=eps_col[0:C, 0:1])
    rc_c = sm.tile([C, 1], F32, tag="c9")
    nc.vector.reciprocal(rc_c[:], sd_c[:])
    isc = sm.tile([C, 1], F32, tag="c10")
    nc.vector.tensor_tensor(isc[:], rc_c[:], msin_w_col[:], OP.mult)
    mi = sm.tile([C, 1], F32, tag="c11")
    nc.vector.tensor_tensor(mi[:], mu_c[:], isc[:], OP.mult)
    ish = sm.tile([C, 1], F32, tag="c12")
    nc.vector.tensor_tensor(ish[:], msin_b_col[:], mi[:], OP.subtract)

    xsum = sm.tile([C, 1], F32, tag="c13")
    nc.vector.tensor_reduce(xsum[:], xbf[:], AX.X, OP.add)
    xmax = sm.tile([C, 1], F32, tag="c14")
    nc.vector.tensor_reduce(xmax[:], xbf[:], AX.X, OP.max)
    pooled = sm.tile([128, 1], F32, tag="c15")
    nc.scalar.mul(pooled[0:C, :], xsum[:], 1.0 / L)
    nc.sync.dma_start(pooled[C:2 * C, :], xmax[:])
    pg = psA.tile([C, 1], F32, tag="a")
    nc.tensor.matmul(pg[:], gate_lhsT[:], pooled[:], start=True, stop=True)
    gate_c = sm.tile([C, 1], F32, tag="c16")
    nc.scalar.activation(gate_c[:], pg[:], AF.Sigmoid, bias=gateb_col[:, 0:1])

    xsp = lb.tile([C, HP * WP], F32, tag="Lb")
    nc.vector.memset(xsp[:], 0.0)
    for rc in range(NRC):
        h0 = rc * RCH
        sl = slice(h0 * W, (h0 + RCH) * W)
        t1 = w2.tile([C, CW], F32, tag="w1")
        nc.scalar.activation(t1[:], xmp[:, W + sl.start:W + sl.stop], AF.Lrelu,
                             scale=isc[:, 0:1], bias=ish[:, 0:1], alpha=0.01)
        nc.vector.scalar_tensor_tensor(
            r3(xsp)[:, 1 + h0:1 + h0 + RCH, 1:W + 1], xbf[:, sl],
            gate_c[:, 0:1], t1[:].rearrange("p (h w) -> p h w", h=RCH),
            OP.mult, OP.add)

    taps = [(0, -1), (0, 1), (-1, 0), (1, 0), (0, 0)]
    for rc in range(NRC):
        h0 = rc * RCH
        pc = psY.tile([C, CW], F32, tag="y")
        for bi, (dh, dw) in enumerate(taps):
            nc.tensor.matmul(pc[:], ax_lhsT[:, C * bi:C * (bi + 1)],
                             r3(xsp)[:, 1 + h0 + dh:1 + h0 + dh + RCH,
                                     1 + dw:1 + dw + W],
                             start=(bi == 0), stop=(bi == len(taps) - 1))
        sk = w2.tile([C, CW], F32, tag="w2")
        nc.scalar.activation(sk[:], pc[:], AF.Relu, scale=bnsc_col[:, 0:1],
                             bias=bnsh_col[:, 0:1])
        nc.sync.dma_start(skip_d[:, h0 * W:(h0 + RCH) * W], sk[:])
        ppw = psB.tile([OC, CW], F32, tag="b")
        nc.tensor.matmul(ppw[:], pw_lhsT[:], sk[:], start=True, stop=True)
        pwsb = w3.tile([OC, CW], F32, tag="w6")
        nc.scalar.activation(pwsb[:], ppw[:], AF.Identity, bias=pwb_col[:, 0:1])
        tmp = w3.tile([OC, CW // 2], F32, tag="w7")
        nc.vector.tensor_tensor(tmp[:], pwsb[:, 0::2], pwsb[:, 1::2], OP.max)
        dn = w3.tile([OC, CW // 4], F32, tag="w8")
        nc.vector.tensor_tensor(
            dn[:].rearrange("p (h w) -> p h w", h=RCH // 2),
            tmp[:].rearrange("p (h w) -> p h w", h=RCH)[:, 0::2, :],
            tmp[:].rearrange("p (h w) -> p h w", h=RCH)[:, 1::2, :], OP.max)
        nc.sync.dma_start(down_d[:, (h0 // 2) * 48:(h0 // 2 + RCH // 2) * 48],
                          dn[:])

    for cm in reversed(pools):
        cm.__exit__(None, None, None)
    tcm.__exit__(None, None, None)
    nc.finalize()
    return nc


def _host_prep(inp):
    f = np.float32
    A = -np.exp(np.asarray(inp['vss_A_logs'], f))
    dt_w = np.einsum('kdr,krc->kdc', np.asarray(inp['vss_dtproj_w'], f),
                     np.asarray(inp['vss_xproj_w'], f)[:, :R])
    bc_w = np.asarray(inp['vss_xproj_w'], f)[:, R:R + 2 * N]
    in_w_fold = (np.asarray(inp['vss_ln_w'], f)[:, None]
                 * np.asarray(inp['vss_in_w'], f))
    in_b_fold = np.asarray(inp['vss_ln_b'], f) @ np.asarray(inp['vss_in_w'], f)
    conv_w = np.asarray(inp['vss_conv_w'], f)[:, 0]
    dw33_w = np.asarray(inp['ms_dw33_w'], f)[:, 0]
    ms_scale = float(np.asarray(inp['ms_scale']).reshape(-1)[0])
    bnsc = (np.asarray(inp['bn_w'], f)
            / np.sqrt(np.asarray(inp['bn_var'], f) + EPS))
    adw_h_w = np.asarray(inp['adw_h_w'], f)[:, 0, :, 0]
    adw_w_w = np.asarray(inp['adw_w_w'], f)[:, 0, 0, :]
    bnsh = (np.asarray(inp['bn_b'], f) - np.asarray(inp['bn_mean'], f) * bnsc
            + bnsc * (np.asarray(inp['adw_h_b'], f)
                      + np.asarray(inp['adw_w_b'], f)))
    gate_w = np.asarray(inp['ma_conv_w'], f)[:, :, 1, 1]
    pw_w = np.asarray(inp['pw_w'], f)[:, :, 0, 0]

    ident = np.eye(128, dtype=f)
    onehot = np.zeros((64, N * 128), f)
    for n in range(N):
        for u in range(2):
            onehot[16 * u + n, 128 * n + 64 * u:128 * n + 64 * (u + 1)] = 1.0
            onehot[32 + 16 * u + n,
                   128 * n + 64 * u:128 * n + 64 * (u + 1)] = 1.0
    ax_lhsT = np.zeros((C, 5 * C), f)
    blocks = [np.diag(adw_w_w[:, 0]), np.diag(adw_w_w[:, 2]),
              np.diag(adw_h_w[:, 0]), np.diag(adw_h_w[:, 2]),
              np.eye(C, dtype=f) + np.diag(adw_h_w[:, 1])
              + np.diag(adw_w_w[:, 1])]
    for bi, blk in enumerate(blocks):
        ax_lhsT[:, C * bi:C * (bi + 1)] = blk
    gate_lhsT = np.concatenate([gate_w[:, :C].T, gate_w[:, C:].T], 0)

    x = np.asarray(inp['x'], f)
    in_maps = []
    for c in range(8):
        b, br, kg = c // 4, (c // 2) % 2, c % 2
        k0, k1 = kg, kg + 2
        chs = slice(HD * br, HD * (br + 1))
        dwl = np.zeros((HD, 9 * HD), f)
        cvl = np.zeros((DI, 9 * DI), f)
        for tap in range(9):
            dh, dw = tap // 3, tap % 3
            dwl[:, HD * tap:HD * (tap + 1)] = np.diag(dw33_w[chs, dh, dw])
            cvl[:, DI * tap:DI * (tap + 1)] = np.diag(conv_w[:, dh, dw])
        m = {
            'xb': x[b].reshape(C, L),
            'xbr': x[b].reshape(C, L)[chs],
            'dw33_lhsT': dwl,
            'dw33b_col': np.asarray(inp['ms_dw33_b'], f)[chs][:, None],
            'in_lhsT': in_w_fold,
            'inb_xp_col': in_b_fold[:DI][:, None],
            'inb_z_col': in_b_fold[DI:][:, None],
            'conv_lhsT': cvl,
            'convb_col': np.asarray(inp['vss_conv_b'], f)[:, None],
            'dt_lhsT': np.concatenate([dt_w[k0].T, dt_w[k1].T], 1),
            'dtb_col': np.concatenate(
                [np.asarray(inp['vss_dtproj_b'], f)[k0],
                 np.asarray(inp['vss_dtproj_b'], f)[k1]])[:, None],
            'bc_lhsT': np.concatenate(
                [bc_w[k0][0:N].T, bc_w[k0][N:2 * N].T,
                 bc_w[k1][0:N].T, bc_w[k1][N:2 * N].T], 1),
            'acols': np.concatenate([A[k0], A[k1]], 0),
            'dmat': np.diag(np.concatenate(
                [np.asarray(inp['vss_Ds'], f)[k0],
                 np.asarray(inp['vss_Ds'], f)[k1]])).astype(f),
            'onehot': onehot,
            'ident': ident,
            'out_lhsT': np.asarray(inp['vss_out_w'], f),
            'onw_col': np.asarray(inp['vss_on_w'], f)[:, None],
            'onb_col': np.asarray(inp['vss_on_b'], f)[:, None],
            'ms1_col': np.full((HD, 1), 1.0 + ms_scale, f),
            'gate_lhsT': gate_lhsT,
            'gateb_col': np.asarray(inp['ma_conv_b'], f)[:, None],
            'ax_lhsT': ax_lhsT,
            'bnsc_col': bnsc[:, None],
            'bnsh_col': bnsh[:, None],
            'pw_lhsT': pw_w.T,
            'pwb_col': np.asarray(inp['pw_b'], f)[:, None],
            'msin_w_col': np.asarray(inp['ms_in_w'], f)[:, None],
            'msin_b_col': np.asarray(inp['ms_in_b'], f)[:, None],
            'sel_col': np.full((128, 1), float(kg), f),
        }
        in_maps.append({k: np.ascontiguousarray(v, f) for k, v in m.items()})
    return in_maps


def kernel(**inputs):
    from concourse.bass_utils import run_bass_kernel_spmd
    if 'nc' not in _CACHE:
        _CACHE['nc'] = _build_nc()
    nc = _CACHE['nc']
    in_maps = _host_prep(inputs)
    res = run_bass_kernel_spmd(nc, in_maps, list(range(8)))
    down = np.zeros((B, OC, 48, 48), np.float32)
    skip = np.zeros((B, C, H, W), np.float32)
    for b in range(2):
        r = res.results[4 * b]
        skip[b] = r['skip_out'].reshape(C, H, W)
        down[b] = r['down_out'].reshape(OC, 48, 48)
    return down, skip


# revision 15
# speedup vs baseline: 1.2957x; 1.2957x over previous
"""Trainium2 Bass kernel for nn_Encoder_Mamba (VMamba VSS encoder block).

Self-contained: kernel(**inputs) -> (down, skip) matching reference.py.

Sharding (8 cores): core c = (b=c//4, br=(c//2)%2, kg=c%2). Each core runs
the full VSS pipeline for branch (b, br), scanning its kg's two directions
(kg=0: row-major fwd+bwd; kg=1: col-major fwd+bwd). Cross-core collectives:
ReduceScatter(add) over pairs {2i,2i+1} (4-direction y sum), AllGather over
{0..3},{4..7} (channel concat). Phase C computed full-H per core; host keeps
cores 0 and 4. kg control differences are SPMD-uniform via 0/1 blend columns.

Selective scan: tensor_tensor_scan (state = dA*state + dBu), partitions =
(2 dirs x 64 ch), one scan per state n (16), chunked over L with carries.
"""
import sys

for _p in ('/opt/trn_rl_repo',):
    if _p not in sys.path:
        sys.path.insert(0, _p)

import numpy as np

B, C, OC, H, W = 2, 64, 128, 96, 96
HD, DI, N, R, K = 32, 64, 16, 2, 4
L = H * W              # 9216
TT = 512
NCH = L // TT          # 18
LH = L // 2            # 4608
NCH2 = LH // TT        # 9
EPS = 1e-5
HP, WP = H + 2, W + 2
RCH = 4
NRC = H // RCH         # 24
CW = RCH * W           # 384
LT = L // 128          # 72
LT2 = LH // 128        # 36

_CACHE = {}


def ts(i, size=TT):
    return slice(i * size, (i + 1) * size)


def _build_nc():
    import concourse.bacc as bacc
    import concourse.tile as tile
    from concourse import mybir
    from concourse.tile import add_dep_helper
    F32 = mybir.dt.float32
    AF = mybir.ActivationFunctionType
    OP = mybir.AluOpType
    AX = mybir.AxisListType

    nc = bacc.Bacc("TRN2", target_bir_lowering=False, debug=False, num_devices=8)
    dp = nc.declare_dram_parameter

    xb_d = dp("xb", [C, L], F32, isOutput=False)
    xbr_d = dp("xbr", [HD, L], F32, isOutput=False)
    dw33_lhsT_d = dp("dw33_lhsT", [HD, 9 * HD], F32, isOutput=False)
    dw33b_col_d = dp("dw33b_col", [HD, 1], F32, isOutput=False)
    in_lhsT_d = dp("in_lhsT", [HD, 2 * DI], F32, isOutput=False)
    inb_xp_col_d = dp("inb_xp_col", [DI, 1], F32, isOutput=False)
    inb_z_col_d = dp("inb_z_col", [DI, 1], F32, isOutput=False)
    conv_lhsT_d = dp("conv_lhsT", [DI, 9 * DI], F32, isOutput=False)
    convb_col_d = dp("convb_col", [DI, 1], F32, isOutput=False)
    dt_lhsT_d = dp("dt_lhsT", [DI, 128], F32, isOutput=False)
    dtb_col_d = dp("dtb_col", [128, 1], F32, isOutput=False)
    bc_lhsT_d = dp("bc_lhsT", [DI, 64], F32, isOutput=False)
    acols_d = dp("acols", [128, N], F32, isOutput=False)
    dmat_d = dp("dmat", [128, 128], F32, isOutput=False)
    onehot_d = dp("onehot", [64, N * 128], F32, isOutput=False)
    ident_d = dp("ident", [128, 128], F32, isOutput=False)
    out_lhsT_d = dp("out_lhsT", [DI, HD], F32, isOutput=False)
    onw_col_d = dp("onw_col", [DI, 1], F32, isOutput=False)
    onb_col_d = dp("onb_col", [DI, 1], F32, isOutput=False)
    ms1_col_d = dp("ms1_col", [HD, 1], F32, isOutput=False)
    gate_lhsT_d = dp("gate_lhsT", [128, C], F32, isOutput=False)
    gateb_col_d = dp("gateb_col", [C, 1], F32, isOutput=False)
    ax_lhsT_d = dp("ax_lhsT", [C, 5 * C], F32, isOutput=False)
    bnsc_col_d = dp("bnsc_col", [C, 1], F32, isOutput=False)
    bnsh_col_d = dp("bnsh_col", [C, 1], F32, isOutput=False)
    pw_lhsT_d = dp("pw_lhsT", [C, OC], F32, isOutput=False)
    pwb_col_d = dp("pwb_col", [OC, 1], F32, isOutput=False)
    msin_w_col_d = dp("msin_w_col", [C, 1], F32, isOutput=False)
    msin_b_col_d = dp("msin_b_col", [C, 1], F32, isOutput=False)
    sel_col_d = dp("sel_col", [128, 1], F32, isOutput=False)
    skip_d = dp("skip_out", [C, L], F32, isOutput=True)
    down_d = dp("down_out", [OC, 48 * 48], F32, isOutput=True)

    scr = nc.dram_tensor("ln_scr", [2, L], F32)       # phase-A LN stats
    scr2 = nc.dram_tensor("ln_scr2", [2, L], F32)     # phase-A LN r/m2
    scr3 = nc.dram_tensor("ln_scr3", [2, LH], F32)    # tail LN stats
    scr4 = nc.dram_tensor("ln_scr4", [2, LH], F32)    # tail LN r/m2
    bc_dr = nc.dram_tensor("bc_dr", [64, L], F32)
    z_half = nc.dram_tensor("z_half", [DI, LH], F32)
    xd_half = nc.dram_tensor("xd_half", [HD, LH], F32)
    rs_in = nc.dram_tensor("rs_in", [2 * DI, LH], F32)
    y_half_dr = nc.dram_tensor("y_half", [DI, LH], F32)
    ag_in = nc.dram_tensor("ag_in", [HD, LH], F32)
    xm_all = nc.dram_tensor("xm_all", [4 * HD, LH], F32)

    pairs = [[0, 1], [2, 3], [4, 5], [6, 7]]
    quads = [[0, 1, 2, 3], [4, 5, 6, 7]]

    tcm = tile.TileContext(nc)
    tc = tcm.__enter__()
    pools = []

    def mkpool(**kw):
        cm = tc.tile_pool(**kw)
        pools.append(cm)
        return cm.__enter__()

    cp = mkpool(name="const", bufs=1)
    lb = mkpool(name="Lb", bufs=3)
    w2 = mkpool(name="w2", bufs=2)
    w3 = mkpool(name="w3", bufs=3)
    sm = mkpool(name="sm", bufs=1)
    psA = mkpool(name="psA", bufs=2, space="PSUM")
    psB = mkpool(name="psB", bufs=2, space="PSUM")
    psY = mkpool(name="psY", bufs=1, space="PSUM")
    psZ = mkpool(name="psZ", bufs=1, space="PSUM")

    def cload(dram, shape):
        t = cp.tile(shape, F32, tag=f"c_{dram.name}")
        nc.sync.dma_start(t[:], dram[:])
        return t

    dw33_lhsT = cload(dw33_lhsT_d, [HD, 9 * HD])
    dw33b_col = cload(dw33b_col_d, [HD, 1])
    in_lhsT = cload(in_lhsT_d, [HD, 2 * DI])
    inb_xp_col = cload(inb_xp_col_d, [DI, 1])
    inb_z_col = cload(inb_z_col_d, [DI, 1])
    conv_lhsT = cload(conv_lhsT_d, [DI, 9 * DI])
    convb_col = cload(convb_col_d, [DI, 1])
    dt_lhsT = cload(dt_lhsT_d, [DI, 128])
    dtb_col = cload(dtb_col_d, [128, 1])
    bc_lhsT = cload(bc_lhsT_d, [DI, 64])
    acols = cload(acols_d, [128, N])
    dmat = cload(dmat_d, [128, 128])
    onehot = cload(onehot_d, [64, N * 128])
    ident = cload(ident_d, [128, 128])
    out_lhsT = cload(out_lhsT_d, [DI, HD])
    onw_col = cload(onw_col_d, [DI, 1])
    onb_col = cload(onb_col_d, [DI, 1])
    ms1_col = cload(ms1_col_d, [HD, 1])
    gate_lhsT = cload(gate_lhsT_d, [128, C])
    gateb_col = cload(gateb_col_d, [C, 1])
    ax_lhsT = cload(ax_lhsT_d, [C, 5 * C])
    bnsc_col = cload(bnsc_col_d, [C, 1])
    bnsh_col = cload(bnsh_col_d, [C, 1])
    pw_lhsT = cload(pw_lhsT_d, [C, OC])
    pwb_col = cload(pwb_col_d, [OC, 1])
    msin_w_col = cload(msin_w_col_d, [C, 1])
    msin_b_col = cload(msin_b_col_d, [C, 1])
    sel_col = cload(sel_col_d, [128, 1])
    ones1_32 = cp.tile([1, HD], F32)
    nc.vector.memset(ones1_32[:], 1.0)
    ones1_64 = cp.tile([1, DI], F32)
    nc.vector.memset(ones1_64[:], 1.0)
    o32_lhsT = cp.tile([HD, 1], F32)
    nc.vector.memset(o32_lhsT[:], 1.0 / HD)
    o64_lhsT = cp.tile([DI, 1], F32)
    nc.vector.memset(o64_lhsT[:], 1.0 / DI)
    carry = cp.tile([128, N], F32)
    nc.vector.memset(carry[:], 0.0)
    eps_col = cp.tile([128, 1], F32)
    nc.vector.memset(eps_col[:], EPS)
    one_col = cp.tile([128, 1], F32)
    nc.vector.memset(one_col[:], 1.0)

    def r3(t, h=HP):
        return t[:].rearrange("p (h w) -> p h w", h=h)

    # ============ PHASE A: dw33 + LN stats ============
    xpad32 = lb.tile([HD, HP * WP], F32, tag="Lb")
    nc.vector.memset(xpad32[:], 0.0)
    xbr_sb = lb.tile([HD, L], F32, tag="Lb")
    nc.sync.dma_start(xbr_sb[:], xbr_d[:])
    nc.scalar.copy(r3(xpad32)[:, 1:H + 1, 1:W + 1], r3(xbr_sb, h=H)[:, :, :])

    xd_sb = lb.tile([HD, L], F32, tag="Lb")
    stat_wr = []
    for rc in range(NRC):
        h0 = rc * RCH
        sl = slice(h0 * W, (h0 + RCH) * W)
        pc = psY.tile([HD, CW], F32, tag="y")
        for tap in range(9):
            dh, dw = tap // 3, tap % 3
            nc.tensor.matmul(pc[:], dw33_lhsT[:, HD * tap:HD * (tap + 1)],
                             r3(xpad32)[:, h0 + dh:h0 + dh + RCH, dw:dw + W],
                             start=(tap == 0), stop=(tap == 8))
        nc.scalar.activation(xd_sb[:, sl], pc[:], AF.Identity,
                             bias=dw33b_col[:, 0:1])
        x2 = w2.tile([HD, CW], F32, tag="w1")
        nc.scalar.activation(x2[:], xd_sb[:, sl], AF.Square)
        pstA = psA.tile([1, CW], F32, tag="a")
        nc.tensor.matmul(pstA[:], o32_lhsT[:], xd_sb[:, sl], start=True,
                         stop=True)
        pstB = psB.tile([1, CW], F32, tag="b")
        nc.tensor.matmul(pstB[:], o32_lhsT[:], x2[:], start=True, stop=True)
        sttA = w2.tile([1, CW], F32, tag="w2")
        nc.scalar.copy(sttA[:], pstA[:])
        sttB = w2.tile([1, CW], F32, tag="w3")
        nc.scalar.copy(sttB[:], pstB[:])
        stat_wr.append(nc.sync.dma_start(scr[0:1, sl], sttA[:]))
        stat_wr.append(nc.sync.dma_start(scr[1:2, sl], sttB[:]))

    # LN math in [128, LT] layout (via DRAM scratch)
    st_t = sm.tile([128, 2 * LT], F32, tag="stt")
    ld0 = nc.sync.dma_start(st_t[:, 0:LT],
                            scr[0:1, :].rearrange("o (p f) -> (o p) f", p=128))
    ld1 = nc.sync.dma_start(st_t[:, LT:2 * LT],
                            scr[1:2, :].rearrange("o (p f) -> (o p) f", p=128))
    for wr in stat_wr:
        add_dep_helper(ld0.ins, wr.ins, reason="scr RAW")
        add_dep_helper(ld1.ins, wr.ins, reason="scr RAW")
    musq = sm.tile([128, LT], F32, tag="t1")
    nc.scalar.activation(musq[:], st_t[:, 0:LT], AF.Square)
    var_t = sm.tile([128, LT], F32, tag="t2")
    nc.vector.tensor_tensor(var_t[:], st_t[:, LT:2 * LT], musq[:], OP.subtract)
    sd_t = sm.tile([128, LT], F32, tag="t3")
    nc.scalar.activation(sd_t[:], var_t[:], AF.Sqrt, bias=eps_col[:, 0:1])
    r_t = sm.tile([128, LT], F32, tag="t4")
    nc.vector.reciprocal(r_t[:], sd_t[:])
    m2_t = sm.tile([128, LT], F32, tag="t5")
    nc.vector.tensor_tensor(m2_t[:], st_t[:, 0:LT], r_t[:], OP.mult)
    w0 = nc.sync.dma_start(scr2[0:1, :].rearrange("o (p f) -> (o p) f", p=128),
                           r_t[:])
    w1_ = nc.sync.dma_start(scr2[1:2, :].rearrange("o (p f) -> (o p) f", p=128),
                            m2_t[:])

    # ============ in_proj -> z + xp(pad) ============
    xpad64 = lb.tile([DI, HP * WP], F32, tag="Lb")
    nc.vector.memset(xpad64[:], 0.0)
    zc_sb = lb.tile([DI, L], F32, tag="Lb")
    for rc in range(NRC):
        h0 = rc * RCH
        sl = slice(h0 * W, (h0 + RCH) * W)
        rr = w2.tile([1, CW], F32, tag="w1")
        mm = w2.tile([1, CW], F32, tag="w2")
        lr = nc.sync.dma_start(rr[:], scr2[0:1, sl])
        lm = nc.sync.dma_start(mm[:], scr2[1:2, sl])
        add_dep_helper(lr.ins, w0.ins, reason="scr2 RAW")
        add_dep_helper(lm.ins, w1_.ins, reason="scr2 RAW")
        rrep = psA.tile([HD, CW], F32, tag="a")
        nc.tensor.matmul(rrep[:], ones1_32[:], rr[:], start=True, stop=True)
        mrep = psB.tile([HD, CW], F32, tag="b")
        nc.tensor.matmul(mrep[:], ones1_32[:], mm[:], start=True, stop=True)
        xh = w3.tile([HD, CW], F32, tag="w6")
        nc.vector.tensor_tensor(xh[:], xd_sb[:, sl], rrep[:], OP.mult)
        nc.vector.tensor_tensor(xh[:], xh[:], mrep[:], OP.subtract)
        pxp = psY.tile([DI, CW], F32, tag="y")
        nc.tensor.matmul(pxp[:], in_lhsT[:, 0:DI], xh[:], start=True, stop=True)
        pzz = psB.tile([DI, CW], F32, tag="b")
        nc.tensor.matmul(pzz[:], in_lhsT[:, DI:2 * DI], xh[:], start=True,
                         stop=True)
        nc.scalar.activation(zc_sb[:, sl], pzz[:], AF.Identity,
                             bias=inb_z_col[:, 0:1])
        nc.scalar.activation(r3(xpad64)[:, 1 + h0:1 + h0 + RCH, 1:W + 1],
                             pxp[:], AF.Identity, bias=inb_xp_col[:, 0:1])

    # spill kg half of z and xd (blend by sel): half = h0 + sel*(h1-h0)
    z_wr, xd_wr = [], []
    for tci in range(NCH2):
        sl = ts(tci)
        t1 = w2.tile([DI, TT], F32, tag="w1")
        nc.vector.tensor_tensor(t1[:], zc_sb[:, LH + tci * TT:LH + (tci + 1) * TT],
                                zc_sb[:, sl], OP.subtract)
        nc.vector.scalar_tensor_tensor(t1[:], t1[:], sel_col[0:DI, 0:1],
                                       zc_sb[:, sl], OP.mult, OP.add)
        z_wr.append(nc.sync.dma_start(z_half[:, sl], t1[:]))
        t2 = w2.tile([HD, TT], F32, tag="w2")
        nc.vector.tensor_tensor(t2[:], xd_sb[:, LH + tci * TT:LH + (tci + 1) * TT],
                                xd_sb[:, sl], OP.subtract)
        nc.vector.scalar_tensor_tensor(t2[:], t2[:], sel_col[0:HD, 0:1],
                                       xd_sb[:, sl], OP.mult, OP.add)
        xd_wr.append(nc.sync.dma_start(xd_half[:, sl], t2[:]))

    # ============ conv3x3 + silu -> xc ============
    xc_sb = lb.tile([DI, L], F32, tag="Lb")
    for rc in range(NRC):
        h0 = rc * RCH
        pc = psY.tile([DI, CW], F32, tag="y")
        for tap in range(9):
            dh, dw = tap // 3, tap % 3
            nc.tensor.matmul(pc[:], conv_lhsT[:, DI * tap:DI * (tap + 1)],
                             r3(xpad64)[:, h0 + dh:h0 + dh + RCH, dw:dw + W],
                             start=(tap == 0), stop=(tap == 8))
        nc.scalar.activation(xc_sb[:, h0 * W:(h0 + RCH) * W], pc[:], AF.Silu,
                             bias=convb_col[:, 0:1])

    # ============ xs_s assembly ============
    xs_s = lb.tile([128, L], F32, tag="Lb")
    for rc in range(NRC):
        sl = slice(rc * CW, (rc + 1) * CW)
        cm = w2.tile([DI, CW], F32, tag="w1")
        # cm chunk: w in [4rc,4rc+4), h 0..95 -> xc[h*96+w]
        nc.vector.tensor_copy(
            cm[:].rearrange("p (w h) -> p w h", w=RCH),
            r3(xc_sb, h=H)[:, :, RCH * rc:RCH * (rc + 1)]
            .rearrange("p h w -> p w h"))
        nc.vector.tensor_tensor(cm[:], cm[:], xc_sb[:, sl], OP.subtract)
        nc.vector.scalar_tensor_tensor(xs_s[0:DI, sl], cm[:],
                                       sel_col[0:DI, 0:1], xc_sb[:, sl],
                                       OP.mult, OP.add)
    for rc in range(NRC):
        sl = slice(rc * CW, (rc + 1) * CW)
        rsl = slice(L - (rc + 1) * CW, L - rc * CW)
        rv = w2.tile([DI, CW], F32, tag="w2")
        nc.vector.tensor_copy(rv[:], xs_s[0:DI, rsl][:, ::-1])
        nc.sync.dma_start(xs_s[DI:128, sl], rv[:])

    # ============ projections: dt_s (SBUF), B/C -> DRAM ============
    dt_s = lb.tile([128, L], F32, tag="Lb")
    bc_wr = [None] * NCH
    for tci in range(NCH):
        sl = ts(tci)
        rsl = slice(L - (tci + 1) * TT, L - tci * TT)
        pdt = psA.tile([128, TT], F32, tag="a")
        nc.tensor.matmul(pdt[:], dt_lhsT[:], xs_s[0:DI, sl], start=True,
                         stop=True)
        edt = w2.tile([128, TT], F32, tag="w3")
        nc.scalar.activation(edt[:], pdt[:], AF.Exp, bias=dtb_col[:, 0:1])
        nc.scalar.activation(dt_s[0:DI, sl], edt[0:DI, :], AF.Ln,
                             bias=one_col[0:DI, 0:1])
        nc.scalar.activation(dt_s[DI:128, rsl][:, ::-1], edt[DI:128, :],
                             AF.Ln, bias=one_col[DI:128, 0:1])
        pbc = psB.tile([64, TT], F32, tag="b")
        nc.tensor.matmul(pbc[:], bc_lhsT[:], xs_s[0:DI, sl], start=True,
                         stop=True)
        bt = w2.tile([64, TT], F32, tag="w1")
        nc.scalar.copy(bt[0:32, :], pbc[0:32, :])
        nc.scalar.copy(bt[32:64, ::-1], pbc[32:64, :])
        wA = nc.sync.dma_start(bc_dr[0:32, sl], bt[0:32, :])
        wB = nc.sync.dma_start(bc_dr[32:64, rsl], bt[32:64, :])
        bc_wr[tci] = (wA, wB)

    # ============ PHASE B: scan ============
    y_all = lb.tile([DI, L], F32, tag="Lb")
    nc.vector.memset(y_all[:], 0.0)
    for ci in range(NCH):
        sl = ts(ci)
        rsl = slice(L - (ci + 1) * TT, L - ci * TT)
        bcc = w2.tile([64, TT], F32, tag="w2")
        lb1 = nc.sync.dma_start(bcc[0:16, :], bc_dr[0:16, sl])
        lb2 = nc.sync.dma_start(bcc[16:32, :], bc_dr[32:48, sl])
        lb3 = nc.sync.dma_start(bcc[32:48, :], bc_dr[16:32, sl])
        lb4 = nc.sync.dma_start(bcc[48:64, :], bc_dr[48:64, sl])
        for ld in (lb1, lb2, lb3, lb4):
            for wrs in (bc_wr[ci], bc_wr[NCH - 1 - ci]):
                for wr in wrs:
                    add_dep_helper(ld.ins, wr.ins, reason="bc RAW")
        dtu = w2.tile([128, TT], F32, tag="w1")
        nc.vector.tensor_tensor(dtu[:], dt_s[:, sl], xs_s[:, sl], OP.mult)
        ysum0 = psY.tile([DI, TT], F32, tag="y0")
        ysum1 = psZ.tile([DI, TT], F32, tag="y1")
        nc.tensor.matmul(ysum0[:], dmat[:, 0:DI], xs_s[:, sl], start=True,
                         stop=False)
        nc.tensor.matmul(ysum1[:], dmat[:, DI:128], xs_s[:, sl], start=True,
                         stop=False)
        for n in range(N):
            dA = w3.tile([128, TT], F32, tag="w6")
            nc.scalar.activation(dA[:], dt_s[:, sl], AF.Exp,
                                 scale=acols[:, n:n + 1])
            bps = psA.tile([128, TT], F32, tag="a")
            nc.tensor.matmul(bps[:], onehot[0:32, 128 * n:128 * (n + 1)],
                             bcc[0:32, :], start=True, stop=True)
            dBu = w3.tile([128, TT], F32, tag="w7")
            nc.vector.tensor_tensor(dBu[:], dtu[:], bps[:], OP.mult)
            hh = w3.tile([128, TT], F32, tag="w8")
            nc.vector.tensor_tensor_scan(hh[:], dA[:], dBu[:],
                                         carry[:, n:n + 1], OP.mult, OP.add)
            nc.scalar.copy(carry[:, n:n + 1], hh[:, TT - 1:TT])
            cps = psB.tile([128, TT], F32, tag="b")
            nc.tensor.matmul(cps[:], onehot[32:64, 128 * n:128 * (n + 1)],
                             bcc[32:64, :], start=True, stop=True)
            hc = w3.tile([128, TT], F32, tag="w9")
            nc.vector.tensor_tensor(hc[:], hh[:], cps[:], OP.mult)
            nc.tensor.matmul(ysum0[:], ident[:, 0:DI], hc[:], start=False,
                             stop=(n == N - 1))
            nc.tensor.matmul(ysum1[:], ident[:, DI:128], hc[:], start=False,
                             stop=(n == N - 1))
        nc.vector.tensor_tensor(y_all[:, sl], y_all[:, sl], ysum0[:], OP.add)
        nc.vector.tensor_tensor(y_all[:, rsl], y_all[:, rsl], ysum1[:, ::-1],
                                OP.add)

    # rs_in assembly: untranspose blend per 4-row chunk
    rs_wr = []
    for rc in range(NRC):
        sl = slice(rc * CW, (rc + 1) * CW)
        t1 = w2.tile([DI, CW], F32, tag="w1")
        nc.vector.tensor_copy(
            t1[:].rearrange("p (h w) -> p h w", h=RCH),
            y_all[:].rearrange("p (w h) -> p h w", w=W)
            [:, RCH * rc:RCH * (rc + 1), :])
        nc.vector.tensor_tensor(t1[:], t1[:], y_all[:, sl], OP.subtract)
        t2 = w2.tile([DI, CW], F32, tag="w2")
        nc.vector.scalar_tensor_tensor(t2[:], t1[:], sel_col[0:DI, 0:1],
                                       y_all[:, sl], OP.mult, OP.add)
        if rc < NRC // 2:
            rs_wr.append(nc.sync.dma_start(rs_in[0:DI, sl], t2[:]))
        else:
            sl2 = slice(rc * CW - LH, (rc + 1) * CW - LH)
            rs_wr.append(nc.sync.dma_start(rs_in[DI:2 * DI, sl2], t2[:]))
    cc1 = nc.gpsimd.collective_compute(
        "ReduceScatter", OP.add, replica_groups=pairs, ins=[rs_in[:]],
        outs=[y_half_dr[:]])

    # ============ B-tail: out-LN + gate + out-proj ============
    yh = lb.tile([DI, LH], F32, tag="Lb")
    lyh = nc.sync.dma_start(yh[:], y_half_dr[:])
    add_dep_helper(lyh.ins, cc1.ins, reason="RS RAW")
    st2_wr = []
    for tci in range(NCH2):
        sl = ts(tci)
        y2 = w2.tile([DI, TT], F32, tag="w1")
        nc.scalar.activation(y2[:], yh[:, sl], AF.Square)
        pstA = psA.tile([1, TT], F32, tag="a")
        nc.tensor.matmul(pstA[:], o64_lhsT[:], yh[:, sl], start=True, stop=True)
        pstB = psB.tile([1, TT], F32, tag="b")
        nc.tensor.matmul(pstB[:], o64_lhsT[:], y2[:], start=True, stop=True)
        stt2A = w2.tile([1, TT], F32, tag="w2")
        nc.scalar.copy(stt2A[:], pstA[:])
        stt2B = w2.tile([1, TT], F32, tag="w3")
        nc.scalar.copy(stt2B[:], pstB[:])
        st2_wr.append(nc.sync.dma_start(scr3[0:1, sl], stt2A[:]))
        st2_wr.append(nc.sync.dma_start(scr3[1:2, sl], stt2B[:]))
    st2t = sm.tile([128, 2 * LT2], F32, tag="stt")
    l20 = nc.sync.dma_start(st2t[:, 0:LT2],
                            scr3[0:1, :].rearrange("o (p f) -> (o p) f", p=128))
    l21 = nc.sync.dma_start(st2t[:, LT2:2 * LT2],
                            scr3[1:2, :].rearrange("o (p f) -> (o p) f", p=128))
    for wr in st2_wr:
        add_dep_helper(l20.ins, wr.ins, reason="scr3 RAW")
        add_dep_helper(l21.ins, wr.ins, reason="scr3 RAW")
    musq2 = sm.tile([128, LT2], F32, tag="t1")
    nc.scalar.activation(musq2[:], st2t[:, 0:LT2], AF.Square)
    var2 = sm.tile([128, LT2], F32, tag="t2")
    nc.vector.tensor_tensor(var2[:], st2t[:, LT2:2 * LT2], musq2[:],
                            OP.subtract)
    sd2 = sm.tile([128, LT2], F32, tag="t3")
    nc.scalar.activation(sd2[:], var2[:], AF.Sqrt, bias=eps_col[:, 0:1])
    r2 = sm.tile([128, LT2], F32, tag="t4")
    nc.vector.reciprocal(r2[:], sd2[:])
    m22 = sm.tile([128, LT2], F32, tag="t5")
    nc.vector.tensor_tensor(m22[:], st2t[:, 0:LT2], r2[:], OP.mult)
    w20 = nc.sync.dma_start(scr4[0:1, :].rearrange("o (p f) -> (o p) f", p=128),
                            r2[:])
    w21 = nc.sync.dma_start(scr4[1:2, :].rearrange("o (p f) -> (o p) f", p=128),
                            m22[:])

    zh2 = lb.tile([DI, LH], F32, tag="Lb")
    lz = nc.sync.dma_start(zh2[:], z_half[:])
    for wr in z_wr:
        add_dep_helper(lz.ins, wr.ins, reason="z RAW")
    xdh2 = lb.tile([HD, LH], F32, tag="Lb")
    lx = nc.sync.dma_start(xdh2[:], xd_half[:])
    for wr in xd_wr:
        add_dep_helper(lx.ins, wr.ins, reason="xd RAW")
    ag_wr = []
    for tci in range(NCH2):
        sl = ts(tci)
        rr = w2.tile([1, TT], F32, tag="w1")
        mm = w2.tile([1, TT], F32, tag="w2")
        lr = nc.sync.dma_start(rr[:], scr4[0:1, sl])
        lm = nc.sync.dma_start(mm[:], scr4[1:2, sl])
        add_dep_helper(lr.ins, w20.ins, reason="scr4 RAW")
        add_dep_helper(lm.ins, w21.ins, reason="scr4 RAW")
        rrep = psA.tile([DI, TT], F32, tag="a")
        nc.tensor.matmul(rrep[:], ones1_64[:], rr[:], start=True, stop=True)
        mrep = psB.tile([DI, TT], F32, tag="b")
        nc.tensor.matmul(mrep[:], ones1_64[:], mm[:], start=True, stop=True)
        t1 = w3.tile([DI, TT], F32, tag="w6")
        nc.vector.tensor_tensor(t1[:], yh[:, sl], rrep[:], OP.mult)
        nc.vector.tensor_tensor(t1[:], t1[:], mrep[:], OP.subtract)
        yl = w3.tile([DI, TT], F32, tag="w7")
        nc.scalar.activation(yl[:], t1[:], AF.Identity, scale=onw_col[:, 0:1],
                             bias=onb_col[:, 0:1])
        sz = w3.tile([DI, TT], F32, tag="w8")
        nc.scalar.activation(sz[:], zh2[:, sl], AF.Silu)
        nc.vector.tensor_tensor(yl[:], yl[:], sz[:], OP.mult)
        p32 = psY.tile([HD, TT], F32, tag="y")
        nc.tensor.matmul(p32[:], out_lhsT[:], yl[:], start=True, stop=True)
        piece = w3.tile([HD, TT], F32, tag="w9")
        nc.vector.scalar_tensor_tensor(piece[:], xdh2[:, sl], ms1_col[:, 0:1],
                                       p32[:], OP.mult, OP.add)
        ag_wr.append(nc.sync.dma_start(ag_in[:, sl], piece[:]))
    cc2 = nc.gpsimd.collective_compute(
        "AllGather", OP.bypass, replica_groups=quads, ins=[ag_in[:]],
        outs=[xm_all[:]])

    # ============ PHASE C (full-H, duplicated per b) ============
    xmp = lb.tile([C, (H + 2) * W], F32, tag="Lb")
    nc.vector.memset(xmp[:], 0.0)
    for rr_ in range(4):
        br, kg = (rr_ // 2) % 2, rr_ % 2
        lxm = nc.sync.dma_start(
            xmp[HD * br:HD * (br + 1), W + kg * LH:W + (kg + 1) * LH],
            xm_all[HD * rr_:HD * (rr_ + 1), :])
        add_dep_helper(lxm.ins, cc2.ins, reason="AG RAW")
    xbf = lb.tile([C, L], F32, tag="Lb")
    nc.sync.dma_start(xbf[:], xb_d[:])

    ssum = sm.tile([C, 1], F32, tag="c1")
    nc.vector.tensor_reduce(ssum[:], xmp[:, W:W + L], AX.X, OP.add)
    ssq = sm.tile([C, 1], F32, tag="c2")
    for qq in range(NCH):
        scr_t = w2.tile([C, TT], F32, tag="w1")
        if qq:
            part = sm.tile([C, 1], F32, tag="c3")
        else:
            part = ssq
        nc.scalar.activation(scr_t[:], xmp[:, W + qq * TT:W + (qq + 1) * TT],
                             AF.Square, accum_out=part[:, 0:1])
        if qq:
            nc.vector.tensor_tensor(ssq[:], ssq[:], part[:], OP.add)
    mu_c = sm.tile([C, 1], F32, tag="c4")
    nc.scalar.mul(mu_c[:], ssum[:], 1.0 / L)
    ex2 = sm.tile([C, 1], F32, tag="c5")
    nc.scalar.mul(ex2[:], ssq[:], 1.0 / L)
    musq_c = sm.tile([C, 1], F32, tag="c6")
    nc.scalar.activation(musq_c[:], mu_c[:], AF.Square)
    var_c = sm.tile([C, 1], F32, tag="c7")
    nc.vector.tensor_tensor(var_c[:], ex2[:], musq_c[:], OP.subtract)
    sd_c = sm.tile([C, 1], F32, tag="c8")
    nc.scalar.activation(sd_c[:], var_c[:], AF.Sqrt, bias=eps_col[0:C, 0:1])
    rc_c = sm.tile([C, 1], F32, tag="c9")
    nc.vector.reciprocal(rc_c[:], sd_c[:])
    isc = sm.tile([C, 1], F32, tag="c10")
    nc.vector.tensor_tensor(isc[:], rc_c[:], msin_w_col[:], OP.mult)
    mi = sm.tile([C, 1], F32, tag="c11")
    nc.vector.tensor_tensor(mi[:], mu_c[:], isc[:], OP.mult)
    ish = sm.tile([C, 1], F32, tag="c12")
    nc.vector.tensor_tensor(ish[:], msin_b_col[:], mi[:], OP.subtract)

    xsum = sm.tile([C, 1], F32, tag="c13")
    nc.vector.tensor_reduce(xsum[:], xbf[:], AX.X, OP.add)
    xmax = sm.tile([C, 1], F32, tag="c14")
    nc.vector.tensor_reduce(xmax[:], xbf[:], AX.X, OP.max)
    pooled = sm.tile([128, 1], F32, tag="c15")
    nc.scalar.mul(pooled[0:C, :], xsum[:], 1.0 / L)
    nc.sync.dma_start(pooled[C:2 * C, :], xmax[:])
    pg = psA.tile([C, 1], F32, tag="a")
    nc.tensor.matmul(pg[:], gate_lhsT[:], pooled[:], start=True, stop=True)
    gate_c = sm.tile([C, 1], F32, tag="c16")
    nc.scalar.activation(gate_c[:], pg[:], AF.Sigmoid, bias=gateb_col[:, 0:1])

    xsp = lb.tile([C, HP * WP], F32, tag="Lb")
    nc.vector.memset(xsp[:], 0.0)
    for rc in range(NRC):
        h0 = rc * RCH
        sl = slice(h0 * W, (h0 + RCH) * W)
        t1 = w2.tile([C, CW], F32, tag="w1")
        nc.scalar.activation(t1[:], xmp[:, W + sl.start:W + sl.stop], AF.Lrelu,
                             scale=isc[:, 0:1], bias=ish[:, 0:1], alpha=0.01)
        nc.vector.scalar_tensor_tensor(
            r3(xsp)[:, 1 + h0:1 + h0 + RCH, 1:W + 1], xbf[:, sl],
            gate_c[:, 0:1], t1[:].rearrange("p (h w) -> p h w", h=RCH),
            OP.mult, OP.add)

    taps = [(0, -1), (0, 1), (-1, 0), (1, 0), (0, 0)]
    for rc in range(NRC):
        h0 = rc * RCH
        pc = psY.tile([C, CW], F32, tag="y")
        for bi, (dh, dw) in enumerate(taps):
            nc.tensor.matmul(pc[:], ax_lhsT[:, C * bi:C * (bi + 1)],
                             r3(xsp)[:, 1 + h0 + dh:1 + h0 + dh + RCH,
                                     1 + dw:1 + dw + W],
                             start=(bi == 0), stop=(bi == len(taps) - 1))
        sk = w2.tile([C, CW], F32, tag="w2")
        nc.scalar.activation(sk[:], pc[:], AF.Relu, scale=bnsc_col[:, 0:1],
                             bias=bnsh_col[:, 0:1])
        nc.sync.dma_start(skip_d[:, h0 * W:(h0 + RCH) * W], sk[:])
        ppw = psB.tile([OC, CW], F32, tag="b")
        nc.tensor.matmul(ppw[:], pw_lhsT[:], sk[:], start=True, stop=True)
        pwsb = w3.tile([OC, CW], F32, tag="w6")
        nc.scalar.activation(pwsb[:], ppw[:], AF.Identity, bias=pwb_col[:, 0:1])
        tmp = w3.tile([OC, CW // 2], F32, tag="w7")
        nc.vector.tensor_tensor(tmp[:], pwsb[:, 0::2], pwsb[:, 1::2], OP.max)
        dn = w3.tile([OC, CW // 4], F32, tag="w8")
        nc.vector.tensor_tensor(
            dn[:].rearrange("p (h w) -> p h w", h=RCH // 2),
            tmp[:].rearrange("p (h w) -> p h w", h=RCH)[:, 0::2, :],
            tmp[:].rearrange("p (h w) -> p h w", h=RCH)[:, 1::2, :], OP.max)
        nc.sync.dma_start(down_d[:, (h0 // 2) * 48:(h0 // 2 + RCH // 2) * 48],
                          dn[:])

    for cm in reversed(pools):
        cm.__exit__(None, None, None)
    tcm.__exit__(None, None, None)
    nc.finalize()
    return nc


def _host_prep(inp):
    f = np.float32
    A = -np.exp(np.asarray(inp['vss_A_logs'], f))
    dt_w = np.einsum('kdr,krc->kdc', np.asarray(inp['vss_dtproj_w'], f),
                     np.asarray(inp['vss_xproj_w'], f)[:, :R])
    bc_w = np.asarray(inp['vss_xproj_w'], f)[:, R:R + 2 * N]
    in_w_fold = (np.asarray(inp['vss_ln_w'], f)[:, None]
                 * np.asarray(inp['vss_in_w'], f))
    in_b_fold = np.asarray(inp['vss_ln_b'], f) @ np.asarray(inp['vss_in_w'], f)
    conv_w = np.asarray(inp['vss_conv_w'], f)[:, 0]
    dw33_w = np.asarray(inp['ms_dw33_w'], f)[:, 0]
    ms_scale = float(np.asarray(inp['ms_scale']).reshape(-1)[0])
    bnsc = (np.asarray(inp['bn_w'], f)
            / np.sqrt(np.asarray(inp['bn_var'], f) + EPS))
    adw_h_w = np.asarray(inp['adw_h_w'], f)[:, 0, :, 0]
    adw_w_w = np.asarray(inp['adw_w_w'], f)[:, 0, 0, :]
    bnsh = (np.asarray(inp['bn_b'], f) - np.asarray(inp['bn_mean'], f) * bnsc
            + bnsc * (np.asarray(inp['adw_h_b'], f)
                      + np.asarray(inp['adw_w_b'], f)))
    gate_w = np.asarray(inp['ma_conv_w'], f)[:, :, 1, 1]
    pw_w = np.asarray(inp['pw_w'], f)[:, :, 0, 0]

    ident = np.eye(128, dtype=f)
    onehot = np.zeros((64, N * 128), f)
    for n in range(N):
        for u in range(2):
            onehot[16 * u + n, 128 * n + 64 * u:128 * n + 64 * (u + 1)] = 1.0
            onehot[32 + 16 * u + n,
                   128 * n + 64 * u:128 * n + 64 * (u + 1)] = 1.0
    ax_lhsT = np.zeros((C, 5 * C), f)
    blocks = [np.diag(adw_w_w[:, 0]), np.diag(adw_w_w[:, 2]),
              np.diag(adw_h_w[:, 0]), np.diag(adw_h_w[:, 2]),
              np.eye(C, dtype=f) + np.diag(adw_h_w[:, 1])
              + np.diag(adw_w_w[:, 1])]
    for bi, blk in enumerate(blocks):
        ax_lhsT[:, C * bi:C * (bi + 1)] = blk
    gate_lhsT = np.concatenate([gate_w[:, :C].T, gate_w[:, C:].T], 0)

    x = np.asarray(inp['x'], f)
    in_maps = []
    for c in range(8):
        b, br, kg = c // 4, (c // 2) % 2, c % 2
        k0, k1 = kg, kg + 2
        chs = slice(HD * br, HD * (br + 1))
        dwl = np.zeros((HD, 9 * HD), f)
        cvl = np.zeros((DI, 9 * DI), f)
        for tap in range(9):
            dh, dw = tap // 3, tap % 3
            dwl[:, HD * tap:HD * (tap + 1)] = np.diag(dw33_w[chs, dh, dw])
            cvl[:, DI * tap:DI * (tap + 1)] = np.diag(conv_w[:, dh, dw])
        m = {
            'xb': x[b].reshape(C, L),
            'xbr': x[b].reshape(C, L)[chs],
            'dw33_lhsT': dwl,
            'dw33b_col': np.asarray(inp['ms_dw33_b'], f)[chs][:, None],
            'in_lhsT': in_w_fold,
            'inb_xp_col': in_b_fold[:DI][:, None],
            'inb_z_col': in_b_fold[DI:][:, None],
            'conv_lhsT': cvl,
            'convb_col': np.asarray(inp['vss_conv_b'], f)[:, None],
            'dt_lhsT': np.concatenate([dt_w[k0].T, dt_w[k1].T], 1),
            'dtb_col': np.concatenate(
                [np.asarray(inp['vss_dtproj_b'], f)[k0],
                 np.asarray(inp['vss_dtproj_b'], f)[k1]])[:, None],
            'bc_lhsT': np.concatenate(
                [bc_w[k0][0:N].T, bc_w[k0][N:2 * N].T,
                 bc_w[k1][0:N].T, bc_w[k1][N:2 * N].T], 1),
            'acols': np.concatenate([A[k0], A[k1]], 0),
            'dmat': np.diag(np.concatenate(
                [np.asarray(inp['vss_Ds'], f)[k0],
                 np.asarray(inp['vss_Ds'], f)[k1]])).astype(f),
            'onehot': onehot,
            'ident': ident,
            'out_lhsT': np.asarray(inp['vss_out_w'], f),
            'onw_col': np.asarray(inp['vss_on_w'], f)[:, None],
            'onb_col': np.asarray(inp['vss_on_b'], f)[:, None],
            'ms1_col': np.full((HD, 1), 1.0 + ms_scale, f),
            'gate_lhsT': gate_lhsT,
            'gateb_col': np.asarray(inp['ma_conv_b'], f)[:, None],
            'ax_lhsT': ax_lhsT,
            'bnsc_col': bnsc[:, None],
            'bnsh_col': bnsh[:, None],
            'pw_lhsT': pw_w.T,
            'pwb_col': np.asarray(inp['pw_b'], f)[:, None],
            'msin_w_col': np.asarray(inp['ms_in_w'], f)[:, None],
            'msin_b_col': np.asarray(inp['ms_in_b'], f)[:, None],
            'sel_col': np.full((128, 1), float(kg), f),
        }
        in_maps.append({k: np.ascontiguousarray(v, f) for k, v in m.items()})
    return in_maps


def kernel(**inputs):
    from concourse.bass_utils import run_bass_kernel_spmd
    if 'nc' not in _CACHE:
        _CACHE['nc'] = _build_nc()
    nc = _CACHE['nc']
    in_maps = _host_prep(inputs)
    res = run_bass_kernel_spmd(nc, in_maps, list(range(8)))
    down = np.zeros((B, OC, 48, 48), np.float32)
    skip = np.zeros((B, C, H, W), np.float32)
    for b in range(2):
        r = res.results[4 * b]
        skip[b] = r['skip_out'].reshape(C, H, W)
        down[b] = r['down_out'].reshape(OC, 48, 48)
    return down, skip
